# revision 1
# baseline (speedup 1.0000x reference)
"""JPEG encoder Bass kernel for TRN2 — self-contained, 8-core data-parallel.

kernel(img, D, Q) -> (flatten, no_quan_flatten), matching the reference:
    per 8x8 block: dct = D @ (X - 128) @ D.T ; quant = round(dct / Q);
    both zigzag-gathered + channel-concatenated to (256, 512, 192).

Design: the whole per-block pipeline is one linear map on the flattened
64-pixel block, folded into fp32r matmuls with matrix
M = kron(D, D)[zigzag, :] (and M / q_zz for the quant path; round done as
(x + 1.5*2^23) - 1.5*2^23 on the vector engine).

Per-core (64 batches): strip DMA loads (contiguous rows) -> regroup pass
(free-dim permute (i,bw,j)->(bw,i,j), subtract 128, round to fp32r) ->
PE transposes (64-wide) to put pixels on partitions -> two fp32r matmuls
(K=128 c0|c1 stacked + K=64 c2, N=384 = [nq 192 | q-preround 192]) ->
copy/round -> contiguous (2, 64, 192) output-block DMAs.
"""

import numpy as np
import concourse.mybir as mybir
import concourse.tile as tile
from concourse import bacc
from concourse.bass_utils import run_bass_kernel_spmd
from concourse.masks import make_identity

F32 = mybir.dt.float32
F32R = mybir.dt.float32r
MAGIC = 12582912.0  # 1.5 * 2**23
P = 8
B, C, H, W = 512, 3, 128, 128
NCORES = 8
BSH = B // NCORES          # 64 batches per core
N = (H // P) * (W // P)    # 256 blocks per plane
CZ = C * P * P             # 192


def _zigzag_flat_idx(n=P):
    order = []
    for s in range(2 * n - 1):
        cells = [(r, s - r) for r in range(max(0, s - n + 1), min(s, n - 1) + 1)]
        if s % 2 == 0:
            cells.reverse()
        order.extend(cells)
    return np.array([r * n + c for r, c in order], dtype=np.int32)


def _build_rhs(D: np.ndarray, Q: np.ndarray):
    ZZ = _zigzag_flat_idx()
    D64 = D.astype(np.float64)
    KD = np.kron(D64, D64)[ZZ, :]          # (64 zz, 64 pix)
    q_zz = Q.astype(np.float64).flatten()[ZZ]
    KDq = KD / q_zz[:, None]
    Mt = KD.T.astype(np.float32)           # (64 pix, 64 zz)
    Mqt = KDq.T.astype(np.float32)
    rhs01 = np.zeros((128, 384), dtype=np.float32)
    rhs2 = np.zeros((64, 384), dtype=np.float32)
    for c, r in ((0, rhs01), (1, rhs01), (2, rhs2)):
        p0 = 64 if c == 1 else 0
        r[p0:p0 + 64, c * 64:(c + 1) * 64] = Mt
        r[p0:p0 + 64, 192 + c * 64:192 + (c + 1) * 64] = Mqt
    return rhs01, rhs2


def _build_nc():
    nc = bacc.Bacc("TRN2", target_bir_lowering=False, debug=False)

    img = nc.dram_tensor("img", [BSH, C, H, W], F32, kind="ExternalInput")
    rhs01 = nc.dram_tensor("rhs01", [128, 384], F32, kind="ExternalInput")
    rhs2 = nc.dram_tensor("rhs2", [64, 384], F32, kind="ExternalInput")
    nq = nc.dram_tensor("nq", [N, BSH, CZ], F32, kind="ExternalOutput")
    qq = nc.dram_tensor("qq", [N, BSH, CZ], F32, kind="ExternalOutput")

    AddOp = mybir.AluOpType.add
    SubOp = mybir.AluOpType.subtract
    Copy = mybir.ActivationFunctionType.Copy

    imgv = img[:].rearrange(
        "b c (bp brp i) w -> c bp brp b (i w)", brp=2, i=P
    )

    with tile.TileContext(nc) as tc:
        with (
            tc.tile_pool(name="const", bufs=1) as constp,
            tc.tile_pool(name="sload", bufs=2) as sload,
            tc.tile_pool(name="greg", bufs=2) as greg,
            tc.tile_pool(name="xt", bufs=4) as xtp,
            tc.tile_pool(name="outs", bufs=4) as outp,
            tc.tile_pool(name="psx", bufs=4, space="PSUM") as psx,
            tc.tile_pool(name="pso", bufs=4, space="PSUM") as pso,
        ):
            r01 = constp.tile([128, 384], F32)
            r2 = constp.tile([64, 384], F32)
            nc.sync.dma_start(out=r01[:], in_=rhs01[:])
            nc.sync.dma_start(out=r2[:], in_=rhs2[:])
            r01r = constp.tile([128, 384], F32R)
            r2r = constp.tile([64, 384], F32R)
            nc.vector.tensor_copy(r01r[:], r01[:])
            nc.vector.tensor_copy(r2r[:], r2[:])
            ident = constp.tile([128, 64], F32)
            make_identity(nc, ident[0:64, :])
            make_identity(nc, ident[64:128, :])
            identf = constp.tile([128, 64], F32R)
            nc.vector.tensor_copy(identf[:], ident[:])
            identr = [identf[0:64, :], identf[64:128, :]]

            for bp in range(8):  # row-block pair index
                S = [sload.tile([128, 1024], F32, tag=f"s{c}", name=f"s{c}")
                     for c in range(3)]
                for c in range(3):
                    for brp in range(2):
                        nc.sync.dma_start(
                            out=S[c][brp * 64:(brp + 1) * 64, :],
                            in_=imgv[c, bp, brp],
                        )
                G = [greg.tile([128, 1024], F32R, tag=f"g{c}", name=f"g{c}")
                     for c in range(3)]
                for c in range(3):
                    sv = S[c][:].rearrange("p (i w j) -> p w i j", i=P, w=16, j=P)
                    gv = G[c][:].rearrange("p (w i j) -> p w i j", i=P, w=16, j=P)
                    nc.vector.tensor_scalar(gv, sv, -128.0, None, AddOp)

                for brp in range(2):
                    br = bp * 2 + brp
                    stnq = outp.tile([128, 1536], F32, tag="stnq", name="stnq")
                    stq = outp.tile([128, 1536], F32, tag="stq", name="stq")
                    for bwp in range(8):
                        pxt = psx.tile([64, 384], F32R)
                        for c in range(3):
                            for k in range(2):
                                bw = bwp * 2 + k
                                nc.tensor.transpose(
                                    pxt[:, (c * 2 + k) * 64:(c * 2 + k + 1) * 64],
                                    G[c][brp * 64:(brp + 1) * 64,
                                         bw * 64:(bw + 1) * 64],
                                    identr[brp],
                                )
                        xA = xtp.tile([128, 128], F32R, tag="xa")
                        xB = xtp.tile([64, 128], F32R, tag="xb")
                        nc.scalar.activation(xA[0:64, :], pxt[:, 0:128], Copy)
                        nc.scalar.activation(xA[64:128, :], pxt[:, 128:256], Copy)
                        nc.scalar.activation(xB[:, :], pxt[:, 256:384], Copy)

                        po = pso.tile([128, 384], F32)
                        nc.tensor.matmul(po[:], xA[:], r01r[:], start=True, stop=False)
                        nc.tensor.matmul(po[:], xB[:], r2r[:], start=False, stop=True)

                        nqs = stnq[:, bwp * 192:(bwp + 1) * 192]
                        qs = stq[:, bwp * 192:(bwp + 1) * 192]
                        nc.scalar.activation(nqs, po[:, 0:192], Copy)
                        nc.vector.tensor_scalar(
                            qs, po[:, 192:384], MAGIC, MAGIC, AddOp, SubOp
                        )

                    dv_nq = nq[br * 16:(br + 1) * 16].rearrange(
                        "(bwp k) b f -> (k b) bwp f", k=2)
                    dv_qq = qq[br * 16:(br + 1) * 16].rearrange(
                        "(bwp k) b f -> (k b) bwp f", k=2)
                    nc.sync.dma_start(
                        out=dv_nq, in_=stnq[:].rearrange("p (bwp f) -> p bwp f", f=192))
                    nc.sync.dma_start(
                        out=dv_qq, in_=stq[:].rearrange("p (bwp f) -> p bwp f", f=192))

    nc.compile()
    return nc


_NC_CACHE = None


def _get_nc():
    global _NC_CACHE
    if _NC_CACHE is None:
        _NC_CACHE = _build_nc()
    return _NC_CACHE


def kernel(img, D, Q):
    img = np.ascontiguousarray(np.asarray(img, dtype=np.float32))
    D = np.asarray(D, dtype=np.float32)
    Q = np.asarray(Q, dtype=np.float32)
    rhs01, rhs2 = _build_rhs(D, Q)

    nc = _get_nc()
    in_maps = [
        {"img": img[k * BSH:(k + 1) * BSH], "rhs01": rhs01, "rhs2": rhs2}
        for k in range(NCORES)
    ]
    res = run_bass_kernel_spmd(nc, in_maps, core_ids=list(range(NCORES)))
    flatten = np.concatenate([r["qq"] for r in res.results], axis=1)
    no_quan = np.concatenate([r["nq"] for r in res.results], axis=1)
    return (flatten, no_quan)



# revision 14
# speedup vs baseline: 1.5750x; 1.5750x over previous
"""JPEG encoder Bass kernel for TRN2 — self-contained, 8-core data-parallel.

kernel(img, D, Q) -> (flatten, no_quan_flatten), matching the reference:
    per 8x8 block: dct = D @ (X - 128) @ D.T ; quant = round(dct / Q);
    both zigzag-gathered + channel-concatenated to (256, 512, 192).

Design: the per-block pipeline is one linear map on the flattened 64-pixel
block, folded into f32r matmuls with matrix M = kron(D, D)[zigzag, :] (and
M / q_zz for the quant path; the f32->int8 output cast rounds to nearest even,
matching jnp.round; |quant| <= 127 for this input range).

Dataflow per core (64 batches), per row-block-pair bp (8 iters):
  fp16 strip DMA (2KB rows) -> gpsimd regroup to block-contiguous layout ->
  PE transposes with a block-diag identity moving both row-pairs at once
  (128x128, 3 per block-column-pair) -> batched PSUM->SBUF copies with the
  -128 bias fused (scalar engine for c0|c1, vector for c2) -> two f32r
  matmuls per block-column (K=128 c0|c1 + K=64 c2, N=384 = [nq|q]) into
  2-bank PSUM tiles -> bf16 copy (nq, scalar) / int8 round-cast (q, vector)
  -> fat contiguous output DMAs (6KB/3KB runs per partition).

PE instruction count is the binding resource in the cost model (71ns seq
decode per instruction, 2 per transpose): 192 transposes + 256 matmuls.
"""

import numpy as np
import concourse.mybir as mybir
import concourse.tile as tile
from concourse import bacc
from concourse.bass_utils import run_bass_kernel_spmd

F32 = mybir.dt.float32
F32R = mybir.dt.float32r
F16 = mybir.dt.float16
BF16 = mybir.dt.bfloat16
I8 = mybir.dt.int8
P = 8
B, C, H, W = 512, 3, 128, 128
NCORES = 8
BSH = B // NCORES          # 64 batches per core
N = (H // P) * (W // P)    # 256 blocks per plane
CZ = C * P * P             # 192
NBR = H // P               # 16 block rows
NBW = W // P               # 16 block cols


def _zigzag_flat_idx(n=P):
    order = []
    for s in range(2 * n - 1):
        cells = [(r, s - r) for r in range(max(0, s - n + 1), min(s, n - 1) + 1)]
        if s % 2 == 0:
            cells.reverse()
        order.extend(cells)
    return np.array([r * n + c for r, c in order], dtype=np.int32)


def _build_rhs(D: np.ndarray, Q: np.ndarray):
    ZZ = _zigzag_flat_idx()
    D64 = D.astype(np.float64)
    KD = np.kron(D64, D64)[ZZ, :]          # (64 zz, 64 pix)
    q_zz = Q.astype(np.float64).flatten()[ZZ]
    KDq = KD / q_zz[:, None]
    Mt = KD.T.astype(np.float32)           # (64 pix, 64 zz)
    Mqt = KDq.T.astype(np.float32)
    rhs01 = np.zeros((128, 384), dtype=np.float32)
    for c in (0, 1):
        p0 = c * 64
        rhs01[p0:p0 + 64, c * 64:(c + 1) * 64] = Mt
        rhs01[p0:p0 + 64, 192 + c * 64:192 + (c + 1) * 64] = Mqt
    # c2 weights on both partition halves: odd block-columns read the c2
    # pixels from partition base 64 (lhsT/rhs partition bases must match)
    rhs2 = np.zeros((128, 384), dtype=np.float32)
    for p0 in (0, 64):
        rhs2[p0:p0 + 64, 128:192] = Mt
        rhs2[p0:p0 + 64, 192 + 128:384] = Mqt
    return rhs01, rhs2


def _build_nc():
    nc = bacc.Bacc("TRN2", target_bir_lowering=False, debug=False)

    img = nc.dram_tensor("img", [BSH, C, H, W], F16, kind="ExternalInput")
    rhs01 = nc.dram_tensor("rhs01", [128, 384], F32R, kind="ExternalInput")
    rhs2 = nc.dram_tensor("rhs2", [128, 384], F32R, kind="ExternalInput")
    bdid = nc.dram_tensor("bdid", [128, 128], F16, kind="ExternalInput")
    # device layout: [br, b, (bw, c, zz)]; host reassembles to (N, B, CZ)
    nqd = nc.dram_tensor("nqd", [NBR, BSH, 3072], BF16, kind="ExternalOutput")
    qqd = nc.dram_tensor("qqd", [NBR, BSH, 3072], I8, kind="ExternalOutput")

    AddOp = mybir.AluOpType.add
    Copy = mybir.ActivationFunctionType.Copy

    # partition = b, free = (c, (i, w)) with 2KB contiguous fp16 rows
    imgv = img[:].rearrange(
        "b c (bp brp i) w -> bp brp b c (i w)", brp=2, i=P
    )

    with tile.TileContext(nc) as tc:
        with (
            tc.tile_pool(name="const", bufs=1) as constp,
            tc.tile_pool(name="sload", bufs=2) as sload,
            tc.tile_pool(name="greg", bufs=2) as greg,
            tc.tile_pool(name="xa", bufs=2) as xap,
            tc.tile_pool(name="xb", bufs=2) as xbp,
            tc.tile_pool(name="outs", bufs=2) as outp,
            tc.tile_pool(name="pxa", bufs=1, space="PSUM") as pxap,
            tc.tile_pool(name="pxb", bufs=1, space="PSUM") as pxbp,
            tc.tile_pool(name="pso", bufs=2, space="PSUM") as psop,
        ):
            r01r = constp.tile([128, 384], F32R)
            r2r = constp.tile([128, 384], F32R)
            bdf = constp.tile([128, 128], F16)
            nc.sync.dma_start(out=bdf[:], in_=bdid[:])
            nc.sync.dma_start(out=r01r[:], in_=rhs01[:])
            nc.sync.dma_start(out=r2r[:], in_=rhs2[:])

            for bp in range(8):  # row-block pair index
                S = sload.tile([128, 3072], F16, tag="s", name=f"s{bp}")
                for brp in range(2):
                    nc.sync.dma_start(
                        out=S[brp * 64:(brp + 1) * 64, :],
                        in_=imgv[bp, brp],
                    )
                # regroup on the gpsimd engine into per-block-column-pair
                # groups of 6 x 64 pixels: z = (k*2+c) for c0/c1, 4+k for c2
                # (bw = 2q+k); G free offset = q*384 + z*64 + (i*8+j)
                G = greg.tile([128, 3072], F16, tag="g", name=f"g{bp}")
                gv = G[:].rearrange("p (q z i j) -> p z q i j",
                                    q=8, z=6, i=P, j=P)
                sv = S[:].rearrange("p (c i q k j) -> p c k q i j",
                                    c=3, i=P, q=8, k=2, j=P)
                for c in range(3):
                    for k in range(2):
                        z = 4 + k if c == 2 else k * 2 + c
                        nc.gpsimd.tensor_copy(gv[:, z], sv[:, c, k])

                # 3 transposes per block-column pair q, each 128x128 moving
                # both row-pairs at once via the block-diag identity:
                #   z0|z1 -> c0|c1 pixels of bw=2q; z2|z3 -> bw=2q+1;
                #   z4|z5 -> c2 pixels of bw=2q,2q+1
                xA = xap.tile([128, 2048], F32R, tag="xa", name=f"xa{bp}")
                xB = xbp.tile([128, 1024], F32R, tag="xb", name=f"xb{bp}")
                for half in range(2):
                    pxa = pxap.tile([128, 1024], F16, tag=f"pxa{half}",
                                    name=f"pxa{bp}h{half}")
                    for qh in range(4):
                        q = half * 4 + qh
                        for k in range(2):
                            nc.tensor.matmul(
                                pxa[:, (qh * 2 + k) * 128:(qh * 2 + k + 1) * 128],
                                G[:, q * 384 + k * 128:q * 384 + (k + 1) * 128],
                                bdf[:], is_transpose=True,
                            )
                    nc.scalar.activation(
                        xA[:, half * 1024:(half + 1) * 1024], pxa[:],
                        Copy, bias=-128.0,
                    )
                pxb = pxbp.tile([128, 1024], F16, tag="pxb", name=f"pxb{bp}")
                for q in range(8):
                    nc.tensor.matmul(
                        pxb[:, q * 128:(q + 1) * 128],
                        G[:, q * 384 + 256:q * 384 + 384],
                        bdf[:], is_transpose=True,
                    )
                nc.vector.tensor_scalar(xB[:], pxb[:], -128.0, None, AddOp)

                stnq = outp.tile([128, 3072], BF16, tag="stnq", name="stnq")
                stq = outp.tile([128, 3072], I8, tag="stq", name="stq")
                for q in range(8):
                    po = psop.tile([128, 1024], F32)
                    for k in range(2):
                        bw = q * 2 + k
                        nc.tensor.matmul(
                            po[:, k * 512:k * 512 + 384],
                            xA[:, bw * 128:(bw + 1) * 128], r01r[:],
                            start=True, stop=False,
                        )
                        nc.tensor.matmul(
                            po[:, k * 512:k * 512 + 384],
                            xB[k * 64:(k + 1) * 64, q * 128:(q + 1) * 128],
                            r2r[k * 64:(k + 1) * 64, :],
                            start=False, stop=True,
                        )
                    pov = po[:].rearrange("p (k f) -> p k f", k=2)
                    nc.scalar.activation(
                        stnq[:, q * 384:(q + 1) * 384].rearrange(
                            "p (k f) -> p k f", k=2),
                        pov[:, :, 0:192], Copy,
                    )
                    nc.vector.tensor_copy(
                        stq[:, q * 384:(q + 1) * 384].rearrange(
                            "p (k f) -> p k f", k=2),
                        pov[:, :, 192:384],
                    )
                for brp in range(2):
                    br = bp * 2 + brp
                    bsl = slice(brp * 64, (brp + 1) * 64)
                    nc.sync.dma_start(out=nqd[br], in_=stnq[bsl, :])
                    nc.sync.dma_start(out=qqd[br], in_=stq[bsl, :])

    nc.compile()
    return nc


_NC_CACHE = None


def _get_nc():
    global _NC_CACHE
    if _NC_CACHE is None:
        _NC_CACHE = _build_nc()
    return _NC_CACHE


def _unshard(dev_out: np.ndarray) -> np.ndarray:
    # [br, b, (bw c zz)] -> (N, BSH, CZ) with n = br*16 + bw
    a = dev_out.reshape(NBR, BSH, NBW, CZ).astype(np.float32)
    return a.transpose(0, 2, 1, 3).reshape(N, BSH, CZ)


def kernel(img, D, Q):
    img = np.ascontiguousarray(np.asarray(img, dtype=np.float32))
    D = np.asarray(D, dtype=np.float32)
    Q = np.asarray(Q, dtype=np.float32)
    rhs01, rhs2 = _build_rhs(D, Q)
    e64 = np.eye(64, dtype=np.float16)
    z64 = np.zeros((64, 64), dtype=np.float16)
    bdid = np.block([[e64, z64], [z64, e64]]).astype(np.float16)

    img16 = img.astype(np.float16)
    nc = _get_nc()
    in_maps = [
        {"img": img16[k * BSH:(k + 1) * BSH], "rhs01": rhs01, "rhs2": rhs2,
         "bdid": bdid}
        for k in range(NCORES)
    ]
    res = run_bass_kernel_spmd(nc, in_maps, core_ids=list(range(NCORES)))
    flatten = np.concatenate(
        [_unshard(np.asarray(r["qqd"])) for r in res.results], axis=1)
    no_quan = np.concatenate(
        [_unshard(np.asarray(r["nqd"])) for r in res.results], axis=1)
    return (flatten, no_quan)


# revision 15
# speedup vs baseline: 2.0887x; 1.3262x over previous
"""JPEG encoder Bass kernel for TRN2 — self-contained, 8-core data-parallel.

kernel(img, D, Q) -> (flatten, no_quan_flatten), matching the reference:
    per 8x8 block: dct = D @ (X - 128) @ D.T ; quant = round(dct / Q);
    both zigzag-gathered + channel-concatenated to (256, 512, 192).

Design: the per-block pipeline is one linear map on the flattened 64-pixel
block, folded into f32r matmuls with matrix M = kron(D, D)[zigzag, :] (and
M / q_zz for the quant path; the f32->int8 output cast rounds to nearest even,
matching jnp.round; |quant| <= 127 for this input range).

Dataflow per core (64 batches), per row-block-pair bp (8 iters):
  fp16 strip DMA (2KB rows) -> gpsimd regroup to block-contiguous layout ->
  PE transposes with a block-diag identity moving both row-pairs at once
  (128x128, 3 per block-column-pair) -> batched PSUM->SBUF copies with the
  -128 bias fused (scalar engine for c0|c1, vector for c2) -> two f32r
  matmuls per block-column (K=128 c0|c1 + K=64 c2, N=384 = [nq|q]) into
  2-bank PSUM tiles -> bf16 copy (nq, scalar) / int8 round-cast (q, vector)
  -> fat contiguous output DMAs (6KB/3KB runs per partition).

PE instruction count is the binding resource in the cost model (71ns seq
decode per instruction, 2 per transpose): 192 transposes + 256 matmuls.
"""

import numpy as np
import concourse.mybir as mybir
import concourse.tile as tile
from concourse import bacc
from concourse.bass_utils import run_bass_kernel_spmd

F32 = mybir.dt.float32
F32R = mybir.dt.float32r
F16 = mybir.dt.float16
BF16 = mybir.dt.bfloat16
I8 = mybir.dt.int8
P = 8
B, C, H, W = 512, 3, 128, 128
NCORES = 8
BSH = B // NCORES          # 64 batches per core
N = (H // P) * (W // P)    # 256 blocks per plane
CZ = C * P * P             # 192
NBR = H // P               # 16 block rows
NBW = W // P               # 16 block cols


def _zigzag_flat_idx(n=P):
    order = []
    for s in range(2 * n - 1):
        cells = [(r, s - r) for r in range(max(0, s - n + 1), min(s, n - 1) + 1)]
        if s % 2 == 0:
            cells.reverse()
        order.extend(cells)
    return np.array([r * n + c for r, c in order], dtype=np.int32)


def _build_rhs(D: np.ndarray, Q: np.ndarray):
    ZZ = _zigzag_flat_idx()
    D64 = D.astype(np.float64)
    KD = np.kron(D64, D64)[ZZ, :]          # (64 zz, 64 pix)
    q_zz = Q.astype(np.float64).flatten()[ZZ]
    KDq = KD / q_zz[:, None]
    Mt = KD.T.astype(np.float32)           # (64 pix, 64 zz)
    Mqt = KDq.T.astype(np.float32)
    rhs01 = np.zeros((128, 384), dtype=np.float32)
    for c in (0, 1):
        p0 = c * 64
        rhs01[p0:p0 + 64, c * 64:(c + 1) * 64] = Mt
        rhs01[p0:p0 + 64, 192 + c * 64:192 + (c + 1) * 64] = Mqt
    # c2 weights on both partition halves: odd block-columns read the c2
    # pixels from partition base 64 (lhsT/rhs partition bases must match)
    rhs2 = np.zeros((128, 384), dtype=np.float32)
    for p0 in (0, 64):
        rhs2[p0:p0 + 64, 128:192] = Mt
        rhs2[p0:p0 + 64, 192 + 128:384] = Mqt
    return rhs01, rhs2


def _build_nc():
    nc = bacc.Bacc("TRN2", target_bir_lowering=False, debug=False)

    img = nc.dram_tensor("img", [BSH, C, H, W], F16, kind="ExternalInput")
    rhs01 = nc.dram_tensor("rhs01", [128, 384], F32R, kind="ExternalInput")
    rhs2 = nc.dram_tensor("rhs2", [128, 384], F32R, kind="ExternalInput")
    bdid = nc.dram_tensor("bdid", [128, 128], F16, kind="ExternalInput")
    # device layout: [br, b, (bw, c, zz)]; host reassembles to (N, B, CZ)
    nqd = nc.dram_tensor("nqd", [NBR, BSH, 3072], BF16, kind="ExternalOutput")
    qqd = nc.dram_tensor("qqd", [NBR, BSH, 3072], I8, kind="ExternalOutput")

    AddOp = mybir.AluOpType.add
    Copy = mybir.ActivationFunctionType.Copy

    # partition = b, free = (c, (i, w)) with 2KB contiguous fp16 rows
    imgv = img[:].rearrange(
        "b c (bp brp i) w -> bp brp b c (i w)", brp=2, i=P
    )

    with tile.TileContext(nc) as tc:
        with (
            tc.tile_pool(name="const", bufs=1) as constp,
            tc.tile_pool(name="sload", bufs=2) as sload,
            tc.tile_pool(name="greg", bufs=2) as greg,
            tc.tile_pool(name="xa", bufs=2) as xap,
            tc.tile_pool(name="xb", bufs=2) as xbp,
            tc.tile_pool(name="outs", bufs=2) as outp,
            tc.tile_pool(name="px", bufs=2, space="PSUM") as pxp,
            tc.tile_pool(name="pso", bufs=3, space="PSUM") as psop,
        ):
            r01r = constp.tile([128, 384], F32R)
            r2r = constp.tile([128, 384], F32R)
            bdf = constp.tile([128, 128], F16)
            nc.sync.dma_start(out=bdf[:], in_=bdid[:])
            nc.sync.dma_start(out=r01r[:], in_=rhs01[:])
            nc.sync.dma_start(out=r2r[:], in_=rhs2[:])

            def load_s(bp):
                S = sload.tile([128, 3072], F16, tag="s", name=f"s{bp}")
                for brp in range(2):
                    nc.sync.dma_start(
                        out=S[brp * 64:(brp + 1) * 64, :],
                        in_=imgv[bp, brp],
                    )
                return S

            S_next = load_s(0)
            for bp in range(8):  # row-block pair index
                S = S_next
                # regroup on the gpsimd engine into per-block-column-pair
                # groups of 6 x 64 pixels: z = (k*2+c) for c0/c1, 4+k for c2
                # (bw = 2q+k); G free offset = q*384 + z*64 + (i*8+j)
                G = greg.tile([128, 3072], F16, tag="g", name=f"g{bp}")
                gv = G[:].rearrange("p (q z i j) -> p z q i j",
                                    q=8, z=6, i=P, j=P)
                sv = S[:].rearrange("p (c i q k j) -> p c k q i j",
                                    c=3, i=P, q=8, k=2, j=P)
                for c, k in ((0, 0), (1, 0), (0, 1), (1, 1), (2, 0), (2, 1)):
                    z = 4 + k if c == 2 else k * 2 + c
                    nc.gpsimd.tensor_copy(gv[:, z], sv[:, c, k])
                # prefetch next strip: keeps the SP DMA queue ahead of the
                # output DMAs below (in-order issue head-of-line)
                if bp < 7:
                    S_next = load_s(bp + 1)

                # 3 transposes per block-column pair q, each 128x128 moving
                # both row-pairs at once via the block-diag identity; the
                # 1-bank PSUM ring holds 8 transposes per tile
                xA = xap.tile([128, 2048], F32R, tag="xa", name=f"xa{bp}")
                xB = xbp.tile([128, 1024], F32R, tag="xb", name=f"xb{bp}")
                for k in range(2):
                    px = pxp.tile([128, 1024], F16, tag="px",
                                  name=f"pxa{bp}k{k}")
                    for q in range(8):
                        nc.tensor.matmul(
                            px[:, q * 128:(q + 1) * 128],
                            G[:, q * 384 + k * 128:q * 384 + (k + 1) * 128],
                            bdf[:], is_transpose=True,
                        )
                    # scatter the 8 same-k transposes to xA slots bw=2q+k
                    nc.scalar.activation(
                        xA[:].rearrange("p (q f) -> p q f", q=16)[:, k::2],
                        px[:].rearrange("p (q f) -> p q f", q=8),
                        Copy, bias=-128.0,
                    )
                px = pxp.tile([128, 1024], F16, tag="px", name=f"pxb{bp}")
                for q in range(8):
                    nc.tensor.matmul(
                        px[:, q * 128:(q + 1) * 128],
                        G[:, q * 384 + 256:q * 384 + 384],
                        bdf[:], is_transpose=True,
                    )
                nc.vector.tensor_scalar(xB[:], px[:], -128.0, None, AddOp)

                stnq = outp.tile([128, 3072], BF16, tag="stnq", name="stnq")
                stq = outp.tile([128, 3072], I8, tag="stq", name="stq")
                for q in range(8):
                    po = psop.tile([128, 1024], F32)
                    for k in range(2):
                        bw = q * 2 + k
                        nc.tensor.matmul(
                            po[:, k * 512:k * 512 + 384],
                            xA[:, bw * 128:(bw + 1) * 128], r01r[:],
                            start=True, stop=False,
                        )
                        nc.tensor.matmul(
                            po[:, k * 512:k * 512 + 384],
                            xB[k * 64:(k + 1) * 64, q * 128:(q + 1) * 128],
                            r2r[k * 64:(k + 1) * 64, :],
                            start=False, stop=True,
                        )
                    pov = po[:].rearrange("p (k f) -> p k f", k=2)
                    nc.scalar.activation(
                        stnq[:, q * 384:(q + 1) * 384].rearrange(
                            "p (k f) -> p k f", k=2),
                        pov[:, :, 0:192], Copy,
                    )
                    nc.vector.tensor_copy(
                        stq[:, q * 384:(q + 1) * 384].rearrange(
                            "p (k f) -> p k f", k=2),
                        pov[:, :, 192:384],
                    )
                for brp in range(2):
                    br = bp * 2 + brp
                    bsl = slice(brp * 64, (brp + 1) * 64)
                    # nq out-DMA issues from the scalar queue (its producer);
                    # q out-DMA stays on SP, behind the next-strip prefetch
                    nc.scalar.dma_start(out=nqd[br], in_=stnq[bsl, :])
                    nc.sync.dma_start(out=qqd[br], in_=stq[bsl, :])

    nc.compile()
    return nc


_NC_CACHE = None


def _get_nc():
    global _NC_CACHE
    if _NC_CACHE is None:
        _NC_CACHE = _build_nc()
    return _NC_CACHE


def _unshard(dev_out: np.ndarray) -> np.ndarray:
    # [br, b, (bw c zz)] -> (N, BSH, CZ) with n = br*16 + bw
    a = dev_out.reshape(NBR, BSH, NBW, CZ).astype(np.float32)
    return a.transpose(0, 2, 1, 3).reshape(N, BSH, CZ)


def kernel(img, D, Q):
    img = np.ascontiguousarray(np.asarray(img, dtype=np.float32))
    D = np.asarray(D, dtype=np.float32)
    Q = np.asarray(Q, dtype=np.float32)
    rhs01, rhs2 = _build_rhs(D, Q)
    e64 = np.eye(64, dtype=np.float16)
    z64 = np.zeros((64, 64), dtype=np.float16)
    bdid = np.block([[e64, z64], [z64, e64]]).astype(np.float16)

    img16 = img.astype(np.float16)
    nc = _get_nc()
    in_maps = [
        {"img": img16[k * BSH:(k + 1) * BSH], "rhs01": rhs01, "rhs2": rhs2,
         "bdid": bdid}
        for k in range(NCORES)
    ]
    res = run_bass_kernel_spmd(nc, in_maps, core_ids=list(range(NCORES)))
    flatten = np.concatenate(
        [_unshard(np.asarray(r["qqd"])) for r in res.results], axis=1)
    no_quan = np.concatenate(
        [_unshard(np.asarray(r["nqd"])) for r in res.results], axis=1)
    return (flatten, no_quan)


# revision 16
# speedup vs baseline: 2.3379x; 1.1193x over previous
"""JPEG encoder Bass kernel for TRN2 — self-contained, 8-core data-parallel.

kernel(img, D, Q) -> (flatten, no_quan_flatten), matching the reference:
    per 8x8 block: dct = D @ (X - 128) @ D.T ; quant = round(dct / Q);
    both zigzag-gathered + channel-concatenated to (256, 512, 192).

Design: the per-block pipeline is one linear map on the flattened 64-pixel
block, folded into f32r matmuls with matrix M = kron(D, D)[zigzag, :] (and
M / q_zz for the quant path; the f32->int8 output cast rounds to nearest even,
matching jnp.round; |quant| <= 127 for this input range).

Dataflow per core (64 batches), per row-block-pair bp (8 iters):
  fp16 strip DMA (2KB rows) -> gpsimd regroup to block-contiguous layout ->
  PE transposes with a block-diag identity moving both row-pairs at once
  (128x128, 3 per block-column-pair) -> batched PSUM->SBUF copies with the
  -128 bias fused (scalar engine for c0|c1, vector for c2) -> two f32r
  matmuls per block-column (K=128 c0|c1 + K=64 c2, N=384 = [nq|q]) into
  2-bank PSUM tiles -> bf16 copy (nq, scalar) / int8 round-cast (q, vector)
  -> fat contiguous output DMAs (6KB/3KB runs per partition).

PE instruction count is the binding resource in the cost model (71ns seq
decode per instruction, 2 per transpose): 192 transposes + 256 matmuls.
"""

import numpy as np
import concourse.mybir as mybir
import concourse.tile as tile
from concourse import bacc
from concourse.bass_utils import run_bass_kernel_spmd

F32 = mybir.dt.float32
F32R = mybir.dt.float32r
F16 = mybir.dt.float16
BF16 = mybir.dt.bfloat16
I8 = mybir.dt.int8
P = 8
B, C, H, W = 512, 3, 128, 128
NCORES = 8
BSH = B // NCORES          # 64 batches per core
N = (H // P) * (W // P)    # 256 blocks per plane
CZ = C * P * P             # 192
NBR = H // P               # 16 block rows
NBW = W // P               # 16 block cols


def _zigzag_flat_idx(n=P):
    order = []
    for s in range(2 * n - 1):
        cells = [(r, s - r) for r in range(max(0, s - n + 1), min(s, n - 1) + 1)]
        if s % 2 == 0:
            cells.reverse()
        order.extend(cells)
    return np.array([r * n + c for r, c in order], dtype=np.int32)


def _build_rhs(D: np.ndarray, Q: np.ndarray):
    ZZ = _zigzag_flat_idx()
    D64 = D.astype(np.float64)
    KD = np.kron(D64, D64)[ZZ, :]          # (64 zz, 64 pix)
    q_zz = Q.astype(np.float64).flatten()[ZZ]
    KDq = KD / q_zz[:, None]
    Mt = KD.T.astype(np.float16)           # (64 pix, 64 zz)
    rhs01 = np.zeros((128, 192), dtype=np.float16)
    for c in (0, 1):
        p0 = c * 64
        rhs01[p0:p0 + 64, c * 64:(c + 1) * 64] = Mt
    # c2 weights on both partition halves: odd block-columns read the c2
    # pixels from partition base 64 (lhsT/rhs partition bases must match)
    rhs2 = np.zeros((128, 192), dtype=np.float16)
    for p0 in (0, 64):
        rhs2[p0:p0 + 64, 128:192] = Mt
    # per-(c,zz) quant reciprocals, repeated for 4 block-columns per PSUM tile
    invq = np.tile((1.0 / q_zz).astype(np.float32), 3)        # (192,)
    invq768 = np.tile(invq, (128, 4))                          # (128, 768)
    return rhs01, rhs2, invq768


def _build_nc():
    nc = bacc.Bacc("TRN2", target_bir_lowering=False, debug=False)

    img = nc.dram_tensor("img", [BSH, C, H, W], F16, kind="ExternalInput")
    rhs01 = nc.dram_tensor("rhs01", [128, 192], F16, kind="ExternalInput")
    rhs2 = nc.dram_tensor("rhs2", [128, 192], F16, kind="ExternalInput")
    invq = nc.dram_tensor("invq", [128, 768], F32, kind="ExternalInput")
    bdid = nc.dram_tensor("bdid", [128, 128], F16, kind="ExternalInput")
    # device layout: [br, b, (bw, c, zz)]; host reassembles to (N, B, CZ)
    nqd = nc.dram_tensor("nqd", [NBR, BSH, 3072], BF16, kind="ExternalOutput")
    qqd = nc.dram_tensor("qqd", [NBR, BSH, 3072], I8, kind="ExternalOutput")

    AddOp = mybir.AluOpType.add
    MulOp = mybir.AluOpType.mult
    Copy = mybir.ActivationFunctionType.Copy

    # partition = b, free = (c, (i, w)) with 2KB contiguous fp16 rows
    imgv = img[:].rearrange(
        "b c (bp brp i) w -> bp brp b c (i w)", brp=2, i=P
    )

    with tile.TileContext(nc) as tc:
        with (
            tc.tile_pool(name="const", bufs=1) as constp,
            tc.tile_pool(name="sload", bufs=2) as sload,
            tc.tile_pool(name="greg", bufs=2) as greg,
            tc.tile_pool(name="xa", bufs=2) as xap,
            tc.tile_pool(name="xb", bufs=2) as xbp,
            tc.tile_pool(name="outs", bufs=2) as outp,
            tc.tile_pool(name="px", bufs=2, space="PSUM") as pxp,
            tc.tile_pool(name="pso", bufs=3, space="PSUM") as psop,
        ):
            r01r = constp.tile([128, 192], F16)
            r2r = constp.tile([128, 192], F16)
            ivq = constp.tile([128, 768], F32)
            bdf = constp.tile([128, 128], F16)
            nc.sync.dma_start(out=bdf[:], in_=bdid[:])
            nc.sync.dma_start(out=r01r[:], in_=rhs01[:])
            nc.sync.dma_start(out=r2r[:], in_=rhs2[:])
            nc.sync.dma_start(out=ivq[:], in_=invq[:])

            def load_s(bp):
                S = sload.tile([128, 3072], F16, tag="s", name=f"s{bp}")
                for brp in range(2):
                    nc.sync.dma_start(
                        out=S[brp * 64:(brp + 1) * 64, :],
                        in_=imgv[bp, brp],
                    )
                return S

            S_next = load_s(0)
            for bp in range(8):  # row-block pair index
                S = S_next
                # regroup on the gpsimd engine into per-block-column-pair
                # groups of 6 x 64 pixels: z = (k*2+c) for c0/c1, 4+k for c2
                # (bw = 2q+k); G free offset = q*384 + z*64 + (i*8+j)
                G = greg.tile([128, 3072], F16, tag="g", name=f"g{bp}")
                gv = G[:].rearrange("p (q z i j) -> p z q i j",
                                    q=8, z=6, i=P, j=P)
                sv = S[:].rearrange("p (c i q k j) -> p c k q i j",
                                    c=3, i=P, q=8, k=2, j=P)
                for c, k in ((0, 0), (1, 0), (0, 1), (1, 1), (2, 0), (2, 1)):
                    z = 4 + k if c == 2 else k * 2 + c
                    nc.gpsimd.tensor_copy(gv[:, z], sv[:, c, k])
                # prefetch next strip: keeps the SP DMA queue ahead of the
                # output DMAs below (in-order issue head-of-line)
                if bp < 7:
                    S_next = load_s(bp + 1)

                # 3 transposes per block-column pair q, each 128x128 moving
                # both row-pairs at once via the block-diag identity; the
                # 1-bank PSUM ring holds 8 transposes per tile
                xA = xap.tile([128, 2048], F16, tag="xa", name=f"xa{bp}")
                xB = xbp.tile([128, 1024], F16, tag="xb", name=f"xb{bp}")
                for k in range(2):
                    px = pxp.tile([128, 1024], F16, tag="px",
                                  name=f"pxa{bp}k{k}")
                    for q in range(8):
                        nc.tensor.matmul(
                            px[:, q * 128:(q + 1) * 128],
                            G[:, q * 384 + k * 128:q * 384 + (k + 1) * 128],
                            bdf[:], is_transpose=True,
                        )
                    # scatter the 8 same-k transposes to xA slots bw=2q+k
                    nc.scalar.activation(
                        xA[:].rearrange("p (q f) -> p q f", q=16)[:, k::2],
                        px[:].rearrange("p (q f) -> p q f", q=8),
                        Copy, bias=-128.0,
                    )
                px = pxp.tile([128, 1024], F16, tag="px", name=f"pxb{bp}")
                for q in range(8):
                    nc.tensor.matmul(
                        px[:, q * 128:(q + 1) * 128],
                        G[:, q * 384 + 256:q * 384 + 384],
                        bdf[:], is_transpose=True,
                    )
                nc.vector.tensor_scalar(xB[:], px[:], -128.0, None, AddOp)

                stnq = outp.tile([128, 3072], BF16, tag="stnq", name="stnq")
                stq = outp.tile([128, 3072], I8, tag="stq", name="stq")
                for g in range(4):  # 4 block-columns per 2-bank PSUM tile
                    po = psop.tile([128, 1024], F32)
                    for s in range(4):
                        bw = g * 4 + s
                        q, k = bw // 2, bw % 2
                        nc.tensor.matmul(
                            po[:, s * 256:s * 256 + 192],
                            xA[:, bw * 128:(bw + 1) * 128], r01r[:],
                            start=True, stop=False,
                        )
                        nc.tensor.matmul(
                            po[:, s * 256:s * 256 + 192],
                            xB[k * 64:(k + 1) * 64, q * 128:(q + 1) * 128],
                            r2r[k * 64:(k + 1) * 64, :],
                            start=False, stop=True,
                        )
                    pov = po[:].rearrange("p (s f) -> p s f", s=4)[:, :, 0:192]
                    nc.scalar.activation(
                        stnq[:, g * 768:(g + 1) * 768].rearrange(
                            "p (s f) -> p s f", s=4),
                        pov, Copy,
                    )
                    # q = rne_int8(nq * (1/Q)): (po * 1.0) * invq, cast to int8
                    nc.vector.scalar_tensor_tensor(
                        stq[:, g * 768:(g + 1) * 768].rearrange(
                            "p (s f) -> p s f", s=4),
                        pov, 1.0,
                        ivq[:].rearrange("p (s f) -> p s f", s=4),
                        MulOp, MulOp,
                    )
                for brp in range(2):
                    br = bp * 2 + brp
                    bsl = slice(brp * 64, (brp + 1) * 64)
                    # nq out-DMA issues from the scalar queue (its producer);
                    # q out-DMA stays on SP, behind the next-strip prefetch
                    nc.scalar.dma_start(out=nqd[br], in_=stnq[bsl, :])
                    nc.sync.dma_start(out=qqd[br], in_=stq[bsl, :])

    nc.compile()
    return nc


_NC_CACHE = None


def _get_nc():
    global _NC_CACHE
    if _NC_CACHE is None:
        _NC_CACHE = _build_nc()
    return _NC_CACHE


def _unshard(dev_out: np.ndarray) -> np.ndarray:
    # [br, b, (bw c zz)] -> (N, BSH, CZ) with n = br*16 + bw
    a = dev_out.reshape(NBR, BSH, NBW, CZ).astype(np.float32)
    return a.transpose(0, 2, 1, 3).reshape(N, BSH, CZ)


def kernel(img, D, Q):
    img = np.ascontiguousarray(np.asarray(img, dtype=np.float32))
    D = np.asarray(D, dtype=np.float32)
    Q = np.asarray(Q, dtype=np.float32)
    rhs01, rhs2, invq768 = _build_rhs(D, Q)
    e64 = np.eye(64, dtype=np.float16)
    z64 = np.zeros((64, 64), dtype=np.float16)
    bdid = np.block([[e64, z64], [z64, e64]]).astype(np.float16)

    img16 = img.astype(np.float16)
    nc = _get_nc()
    in_maps = [
        {"img": img16[k * BSH:(k + 1) * BSH], "rhs01": rhs01, "rhs2": rhs2,
         "bdid": bdid, "invq": invq768}
        for k in range(NCORES)
    ]
    res = run_bass_kernel_spmd(nc, in_maps, core_ids=list(range(NCORES)))
    flatten = np.concatenate(
        [_unshard(np.asarray(r["qqd"])) for r in res.results], axis=1)
    no_quan = np.concatenate(
        [_unshard(np.asarray(r["nqd"])) for r in res.results], axis=1)
    return (flatten, no_quan)


# revision 17
# speedup vs baseline: 2.4063x; 1.0293x over previous
"""JPEG encoder Bass kernel for TRN2 — self-contained, 8-core data-parallel.

kernel(img, D, Q) -> (flatten, no_quan_flatten), matching the reference:
    per 8x8 block: dct = D @ (X - 128) @ D.T ; quant = round(dct / Q);
    both zigzag-gathered + channel-concatenated to (256, 512, 192).

Design: the per-block pipeline is one linear map on the flattened 64-pixel
block, folded into f32r matmuls with matrix M = kron(D, D)[zigzag, :] (and
M / q_zz for the quant path; the f32->int8 output cast rounds to nearest even,
matching jnp.round; |quant| <= 127 for this input range).

Dataflow per core (64 batches), per row-block-pair bp (8 iters):
  fp16 strip DMA (2KB rows) -> gpsimd regroup to block-contiguous layout ->
  PE transposes with a block-diag identity moving both row-pairs at once
  (128x128, 3 per block-column-pair) -> batched PSUM->SBUF copies with the
  -128 bias fused (scalar engine for c0|c1, vector for c2) -> two f32r
  matmuls per block-column (K=128 c0|c1 + K=64 c2, N=384 = [nq|q]) into
  2-bank PSUM tiles -> bf16 copy (nq, scalar) / int8 round-cast (q, vector)
  -> fat contiguous output DMAs (6KB/3KB runs per partition).

PE instruction count is the binding resource in the cost model (71ns seq
decode per instruction, 2 per transpose): 192 transposes + 256 matmuls.
"""

import numpy as np
import concourse.mybir as mybir
import concourse.tile as tile
from concourse import bacc
from concourse.bass_utils import run_bass_kernel_spmd

F32 = mybir.dt.float32
F32R = mybir.dt.float32r
F16 = mybir.dt.float16
BF16 = mybir.dt.bfloat16
I8 = mybir.dt.int8
P = 8
B, C, H, W = 512, 3, 128, 128
NCORES = 8
BSH = B // NCORES          # 64 batches per core
N = (H // P) * (W // P)    # 256 blocks per plane
CZ = C * P * P             # 192
NBR = H // P               # 16 block rows
NBW = W // P               # 16 block cols


def _zigzag_flat_idx(n=P):
    order = []
    for s in range(2 * n - 1):
        cells = [(r, s - r) for r in range(max(0, s - n + 1), min(s, n - 1) + 1)]
        if s % 2 == 0:
            cells.reverse()
        order.extend(cells)
    return np.array([r * n + c for r, c in order], dtype=np.int32)


def _build_rhs(D: np.ndarray, Q: np.ndarray):
    ZZ = _zigzag_flat_idx()
    D64 = D.astype(np.float64)
    KD = np.kron(D64, D64)[ZZ, :]          # (64 zz, 64 pix)
    q_zz = Q.astype(np.float64).flatten()[ZZ]
    KDq = KD / q_zz[:, None]
    Mt = KD.T.astype(np.float16)           # (64 pix, 64 zz)
    rhs01 = np.zeros((128, 192), dtype=np.float16)
    for c in (0, 1):
        p0 = c * 64
        rhs01[p0:p0 + 64, c * 64:(c + 1) * 64] = Mt
    # c2 weights on both partition halves: odd block-columns read the c2
    # pixels from partition base 64 (lhsT/rhs partition bases must match)
    rhs2 = np.zeros((128, 192), dtype=np.float16)
    for p0 in (0, 64):
        rhs2[p0:p0 + 64, 128:192] = Mt
    # per-(c,zz) quant reciprocals, repeated for 4 block-columns per PSUM tile
    invq = np.tile((1.0 / q_zz).astype(np.float32), 3)        # (192,)
    invq768 = np.tile(invq, (128, 4))                          # (128, 768)
    return rhs01, rhs2, invq768


def _build_nc():
    nc = bacc.Bacc("TRN2", target_bir_lowering=False, debug=False)

    img = nc.dram_tensor("img", [BSH, C, H, W], F16, kind="ExternalInput")
    rhs01 = nc.dram_tensor("rhs01", [128, 192], F16, kind="ExternalInput")
    rhs2 = nc.dram_tensor("rhs2", [128, 192], F16, kind="ExternalInput")
    invq = nc.dram_tensor("invq", [128, 768], F32, kind="ExternalInput")
    bdid = nc.dram_tensor("bdid", [128, 128], F16, kind="ExternalInput")
    # device layout: [br, b, (bw, c, zz)]; host reassembles to (N, B, CZ)
    nqd = nc.dram_tensor("nqd", [NBR, BSH, 3072], BF16, kind="ExternalOutput")
    qqd = nc.dram_tensor("qqd", [NBR, BSH, 3072], I8, kind="ExternalOutput")

    AddOp = mybir.AluOpType.add
    MulOp = mybir.AluOpType.mult
    Copy = mybir.ActivationFunctionType.Copy

    # partition = b, free = (c, (i, w)) with 2KB contiguous fp16 rows
    imgv = img[:].rearrange(
        "b c (bp brp i) w -> bp brp b c (i w)", brp=2, i=P
    )

    with tile.TileContext(nc) as tc:
        with (
            tc.tile_pool(name="const", bufs=1) as constp,
            tc.tile_pool(name="sload", bufs=2) as sload,
            tc.tile_pool(name="greg", bufs=2) as greg,
            tc.tile_pool(name="xa", bufs=2) as xap,
            tc.tile_pool(name="xb", bufs=2) as xbp,
            tc.tile_pool(name="outs", bufs=2) as outp,
            tc.tile_pool(name="px", bufs=2, space="PSUM") as pxp,
            tc.tile_pool(name="pso", bufs=3, space="PSUM") as psop,
        ):
            r01r = constp.tile([128, 192], F16)
            r2r = constp.tile([128, 192], F16)
            ivq = constp.tile([128, 768], F32)
            bdf = constp.tile([128, 128], F16)
            nc.sync.dma_start(out=bdf[:], in_=bdid[:])
            nc.sync.dma_start(out=r01r[:], in_=rhs01[:])
            nc.sync.dma_start(out=r2r[:], in_=rhs2[:])
            nc.sync.dma_start(out=ivq[:], in_=invq[:])

            def load_s(bp, fine=False):
                S = sload.tile([128, 3072], F16, tag="s", name=f"s{bp}")
                for brp in range(2):
                    if fine:
                        for c in range(3):
                            nc.sync.dma_start(
                                out=S[brp * 64:(brp + 1) * 64,
                                      c * 1024:(c + 1) * 1024],
                                in_=imgv[bp, brp, :, c],
                            )
                    else:
                        nc.sync.dma_start(
                            out=S[brp * 64:(brp + 1) * 64, :],
                            in_=imgv[bp, brp],
                        )
                return S

            def regroup(S, bp):
                # regroup on the gpsimd engine into per-block-column-pair
                # groups of 6 x 64 pixels: z = (k*2+c) for c0/c1, 4+k for c2
                # (bw = 2q+k); G free offset = q*384 + z*64 + (i*8+j)
                G = greg.tile([128, 3072], F16, tag="g", name=f"g{bp}")
                gv = G[:].rearrange("p (q z i j) -> p z q i j",
                                    q=8, z=6, i=P, j=P)
                sv = S[:].rearrange("p (c i q k j) -> p c k q i j",
                                    c=3, i=P, q=8, k=2, j=P)
                for c, k in ((0, 0), (1, 0), (0, 1), (1, 1), (2, 0), (2, 1)):
                    z = 4 + k if c == 2 else k * 2 + c
                    nc.gpsimd.tensor_copy(gv[:, z], sv[:, c, k])
                return G

            def transpose_stage(G, bp):
                # 3 transposes per block-column pair q, each 128x128 moving
                # both row-pairs at once via the block-diag identity; the
                # 1-bank PSUM ring holds 8 transposes per tile. The -128
                # bias rides on the PSUM->SBUF copies (cpA0 on scalar,
                # cpA1/cpB on vector, which gets 2x throughput on fp16).
                xA = xap.tile([128, 2048], F16, tag="xa", name=f"xa{bp}")
                xB = xbp.tile([128, 1024], F16, tag="xb", name=f"xb{bp}")
                for k in range(2):
                    px = pxp.tile([128, 1024], F16, tag="px",
                                  name=f"pxa{bp}k{k}")
                    for q in range(8):
                        nc.tensor.matmul(
                            px[:, q * 128:(q + 1) * 128],
                            G[:, q * 384 + k * 128:q * 384 + (k + 1) * 128],
                            bdf[:], is_transpose=True,
                        )
                    xav = xA[:].rearrange("p (q f) -> p q f", q=16)[:, k::2]
                    pxv = px[:].rearrange("p (q f) -> p q f", q=8)
                    if k == 0:
                        nc.scalar.activation(xav, pxv, Copy, bias=-128.0)
                    else:
                        nc.vector.tensor_scalar(xav, pxv, -128.0, None, AddOp)
                px = pxp.tile([128, 1024], F16, tag="px", name=f"pxb{bp}")
                for q in range(8):
                    nc.tensor.matmul(
                        px[:, q * 128:(q + 1) * 128],
                        G[:, q * 384 + 256:q * 384 + 384],
                        bdf[:], is_transpose=True,
                    )
                nc.vector.tensor_scalar(xB[:], px[:], -128.0, None, AddOp)
                return xA, xB

            def compute_stage(xA, xB, bp):
                stnq = outp.tile([128, 3072], BF16, tag="stnq", name="stnq")
                stq = outp.tile([128, 3072], I8, tag="stq", name="stq")
                for g in range(4):  # 4 block-columns per 2-bank PSUM tile
                    po = psop.tile([128, 1024], F32)
                    for s in range(4):
                        bw = g * 4 + s
                        q, k = bw // 2, bw % 2
                        nc.tensor.matmul(
                            po[:, s * 256:s * 256 + 192],
                            xA[:, bw * 128:(bw + 1) * 128], r01r[:],
                            start=True, stop=False,
                        )
                        nc.tensor.matmul(
                            po[:, s * 256:s * 256 + 192],
                            xB[k * 64:(k + 1) * 64, q * 128:(q + 1) * 128],
                            r2r[k * 64:(k + 1) * 64, :],
                            start=False, stop=True,
                        )
                    pov = po[:].rearrange("p (s f) -> p s f", s=4)[:, :, 0:192]
                    nc.scalar.activation(
                        stnq[:, g * 768:(g + 1) * 768].rearrange(
                            "p (s f) -> p s f", s=4),
                        pov, Copy,
                    )
                    # q = rne_int8(nq * (1/Q)): (po * 1.0) * invq, cast to int8
                    nc.vector.scalar_tensor_tensor(
                        stq[:, g * 768:(g + 1) * 768].rearrange(
                            "p (s f) -> p s f", s=4),
                        pov, 1.0,
                        ivq[:].rearrange("p (s f) -> p s f", s=4),
                        MulOp, MulOp,
                    )
                halves = 2 if bp == 7 else 1  # finer tail DMAs drain earlier
                for brp in range(2):
                    br = bp * 2 + brp
                    bsl = slice(brp * 64, (brp + 1) * 64)
                    fh = 3072 // halves
                    for h in range(halves):
                        fsl = slice(h * fh, (h + 1) * fh)
                        # nq out-DMA issues from the scalar queue (its
                        # producer); q out-DMA stays on SP, behind the
                        # next-strip prefetch
                        nc.scalar.dma_start(out=nqd[br, :, fsl],
                                            in_=stnq[bsl, fsl])
                        nc.sync.dma_start(out=qqd[br, :, fsl],
                                          in_=stq[bsl, fsl])

            # software-pipelined emission: transposes/copies of bp+1 are
            # emitted BEFORE the matmul/output phase of bp so no engine
            # queue has head-of-line stalls at strip boundaries
            S_cur = load_s(0, fine=True)
            G_cur = regroup(S_cur, 0)
            S_nxt = load_s(1)
            x_cur = transpose_stage(G_cur, 0)
            for bp in range(8):
                if bp + 1 < 8:
                    G_nxt = regroup(S_nxt, bp + 1)
                    if bp + 2 < 8:
                        S_nxt = load_s(bp + 2)
                    x_nxt = transpose_stage(G_nxt, bp + 1)
                compute_stage(x_cur[0], x_cur[1], bp)
                if bp + 1 < 8:
                    x_cur = x_nxt

    nc.compile()
    return nc


_NC_CACHE = None


def _get_nc():
    global _NC_CACHE
    if _NC_CACHE is None:
        _NC_CACHE = _build_nc()
    return _NC_CACHE


def _unshard(dev_out: np.ndarray) -> np.ndarray:
    # [br, b, (bw c zz)] -> (N, BSH, CZ) with n = br*16 + bw
    a = dev_out.reshape(NBR, BSH, NBW, CZ).astype(np.float32)
    return a.transpose(0, 2, 1, 3).reshape(N, BSH, CZ)


def kernel(img, D, Q):
    img = np.ascontiguousarray(np.asarray(img, dtype=np.float32))
    D = np.asarray(D, dtype=np.float32)
    Q = np.asarray(Q, dtype=np.float32)
    rhs01, rhs2, invq768 = _build_rhs(D, Q)
    e64 = np.eye(64, dtype=np.float16)
    z64 = np.zeros((64, 64), dtype=np.float16)
    bdid = np.block([[e64, z64], [z64, e64]]).astype(np.float16)

    img16 = img.astype(np.float16)
    nc = _get_nc()
    in_maps = [
        {"img": img16[k * BSH:(k + 1) * BSH], "rhs01": rhs01, "rhs2": rhs2,
         "bdid": bdid, "invq": invq768}
        for k in range(NCORES)
    ]
    res = run_bass_kernel_spmd(nc, in_maps, core_ids=list(range(NCORES)))
    flatten = np.concatenate(
        [_unshard(np.asarray(r["qqd"])) for r in res.results], axis=1)
    no_quan = np.concatenate(
        [_unshard(np.asarray(r["nqd"])) for r in res.results], axis=1)
    return (flatten, no_quan)
